# revision 31
# baseline (speedup 1.0000x reference)
"""Multi-head attention (B=2, S=2048, D=1024, H=16) on 8 Trainium2 cores.

Sharding: core c -> (batch b = c//4, head-group g = c%4, 4 heads each).
Tensor-parallel over heads within a batch; the output projection is done
per head-group against the matching Wo column slice and the partial
[S, D] results are summed on the host (plus the folded biases bo + Wo@bv).

All matmul operands are bf16 (fp32 PSUM accumulation); exp runs on the
scalar engine in 1024-wide tiles. PV is computed in [q-part, d-free]
orientation (free=65 incl. the ones-column denominator, half the PE cost
of the [d, q] orientation). Normalization is a per-partition reciprocal +
tensor_scalar multiply on DVE; the [q, j] -> [j, q] layout flip for the
output projection is a PE transpose (1 cycle/row) + DVE copy.

Schedule: the exp chain on the scalar engine is the critical resource
(~133us); everything else is arranged to keep it fed: q/k projections
land just before their first scores, K/V projections for later chunks are
spread one-per-iteration through half 0's score/exp loop, q-projections
for half hf+1 run as late fillers inside half hf, and output projections
of half hf-1 fill the scalar-bound middle of half hf.
"""

from contextlib import ExitStack

import numpy as np

import concourse.bacc as bacc
import concourse.tile as tile
from concourse import mybir

D_MODEL = 1024
NUM_HEADS = 16
D_K = 64
B = 2
S_FULL = 2048
N_CORES = 8
GH = 4              # heads per core
GJ = GH * D_K       # 256 columns per head-group

F32 = mybir.dt.float32
BF16 = mybir.dt.bfloat16
AF = mybir.ActivationFunctionType
ALU = mybir.AluOpType


def build_nc(S=S_FULL, SB=512):
    """Build + compile the per-core program (identical on all 8 cores)."""
    assert S % SB == 0
    NCH = S // SB     # chunks / q halves
    ST = S // 128     # sk tiles
    STB = ST // NCH   # sk tiles per chunk
    DT = D_MODEL // 128
    JT = GJ // 128    # 2 j-tiles (2 heads each)
    QT = SB // 128    # q sub-tiles per half
    PEND = 4          # scores/exp run this many st ahead of PV

    nc = bacc.Bacc("TRN2", target_bir_lowering=False, debug=False)

    xqT = nc.dram_tensor("xqT", [NCH, 128, DT, SB], BF16, kind="ExternalInput").ap()
    xkT = nc.dram_tensor("xkT", [NCH, 128, DT, SB], BF16, kind="ExternalInput").ap()
    xvT = nc.dram_tensor("xvT", [NCH, 128, DT, SB], BF16, kind="ExternalInput").ap()
    wqT = nc.dram_tensor("wqT", [128, DT, GJ], BF16, kind="ExternalInput").ap()
    wkT = nc.dram_tensor("wkT", [128, DT, GJ], BF16, kind="ExternalInput").ap()
    wvT = nc.dram_tensor("wvT", [128, DT, GJ], BF16, kind="ExternalInput").ap()
    woT = nc.dram_tensor("woT", [128, GJ // 128, D_MODEL], BF16, kind="ExternalInput").ap()
    ident = nc.dram_tensor("ident", [128, 128], BF16, kind="ExternalInput").ap()
    bq = nc.dram_tensor("bq", [128, GJ // 128], F32, kind="ExternalInput").ap()
    bk = nc.dram_tensor("bk", [128, GJ // 128], F32, kind="ExternalInput").ap()
    yT = nc.dram_tensor("yT", [D_MODEL, S], BF16, kind="ExternalOutput").ap()

    with tile.TileContext(nc) as tc:
        with ExitStack() as ctx:
            cpool = ctx.enter_context(tc.tile_pool(name="const", bufs=1))
            xk_pool = ctx.enter_context(tc.tile_pool(name="xk", bufs=NCH))
            xv_pool = ctx.enter_context(tc.tile_pool(name="xv", bufs=NCH))
            xq_pool = ctx.enter_context(tc.tile_pool(name="xq", bufs=NCH))
            p_pool = ctx.enter_context(tc.tile_pool(name="pt", bufs=PEND + 3))
            y_pool = ctx.enter_context(tc.tile_pool(name="ys", bufs=4))
            s_pool = ctx.enter_context(tc.tile_pool(name="sm", bufs=4))
            o_pool = ctx.enter_context(tc.tile_pool(name="op", bufs=10))
            ps_s = ctx.enter_context(tc.tile_pool(name="ps2", bufs=2, space="PSUM"))
            ps_mix = ctx.enter_context(tc.tile_pool(name="mix", bufs=4, space="PSUM"))

            # ---- persistent SBUF ----
            wq_sb = cpool.tile([128, DT, GJ], BF16, tag="wq")
            wk_sb = cpool.tile([128, DT, GJ], BF16, tag="wk")
            wv_sb = cpool.tile([128, DT, GJ], BF16, tag="wv")
            wo_sb = cpool.tile([128, JT, D_MODEL], BF16, tag="wo")
            id_sb = cpool.tile([128, 128], BF16, tag="ident")
            bq_sb = cpool.tile([128, JT], F32, tag="bq")
            bk_sb = cpool.tile([128, JT], F32, tag="bk")

            qhT_sb = cpool.tile([128, JT, S], BF16, tag="qhT")
            khT_sb = cpool.tile([128, JT, S], BF16, tag="khT")
            vh_sb = cpool.tile([128, ST, GH, 65], BF16, tag="vh")
            oall_sb = cpool.tile([128, JT, S], BF16, tag="oall")

            ones_sb = cpool.tile([128, 1], BF16, tag="ones")
            nc.vector.memset(ones_sb[:], 1.0)
            nc.vector.tensor_copy(
                vh_sb[:, :, :, 64:65],
                ones_sb[:, None, :].broadcast_to([128, ST, GH, 1]),
            )
            # pre-warm the Exp activation table during the input DMA wait
            warm = s_pool.tile([128, 2], F32, tag="warm")
            nc.vector.memset(warm[:], 0.0)
            nc.scalar.activation(warm[:, 1:2], warm[:, 0:1], AF.Exp)

            # ---- input DMAs: all on the SP queue, wait-free, q0/k0/v0 first ----
            xk_sb = [xk_pool.tile([128, DT, SB], BF16, tag="xk", name=f"xk{sb}")
                     for sb in range(NCH)]
            xv_sb = [xv_pool.tile([128, DT, SB], BF16, tag="xv", name=f"xv{sb}")
                     for sb in range(NCH)]
            xq_sb = [xq_pool.tile([128, DT, SB], BF16, tag="xq", name=f"xq{sb}")
                     for sb in range(NCH)]
            nc.sync.dma_start(wq_sb[:], wqT)
            nc.sync.dma_start(xq_sb[0][:], xqT[0])
            nc.sync.dma_start(wk_sb[:], wkT)
            nc.sync.dma_start(xk_sb[0][:], xkT[0])
            nc.sync.dma_start(bq_sb[:], bq)
            nc.sync.dma_start(bk_sb[:], bk)
            nc.sync.dma_start(wv_sb[:], wvT)
            nc.sync.dma_start(xv_sb[0][:], xvT[0])
            for sb in range(1, NCH):
                nc.sync.dma_start(xk_sb[sb][:], xkT[sb])
                nc.sync.dma_start(xv_sb[sb][:], xvT[sb])
                nc.sync.dma_start(xq_sb[sb][:], xqT[sb])
            nc.sync.dma_start(id_sb[:], ident)
            nc.sync.dma_start(wo_sb[:], woT)

            # ---- projection units (each allocs its own ps tile) ----
            def kq_unit(which, sb, jt, pool=None):
                xs, w_sb, b_sb, outT = {
                    "k": (xk_sb[sb], wk_sb, bk_sb, khT_sb),
                    "q": (xq_sb[sb], wq_sb, bq_sb, qhT_sb),
                }[which]

                def u():
                    ss = slice(sb * SB, (sb + 1) * SB)
                    if pool is None:
                        ps = ps_s.tile([128, JT * SB], F32, tag="ps",
                                       name=f"ps_{which}{sb}j{jt}")
                    else:
                        ps = ps_mix.tile([128, SB], F32, tag="mx",
                                         name=f"ps_{which}{sb}j{jt}")
                    for d in range(DT):
                        nc.tensor.matmul(
                            ps[:, 0:SB],
                            w_sb[:, d, jt * 128:(jt + 1) * 128],
                            xs[:, d, :],
                            start=(d == 0),
                            stop=(d == DT - 1),
                        )
                    nc.vector.tensor_scalar_add(
                        outT[:, jt, ss], ps[:, 0:SB], b_sb[:, jt:jt + 1]
                    )
                return u

            def v_unit(sb, stl, pool=None):
                def u():
                    st = sb * STB + stl
                    if pool is None:
                        ps = ps_s.tile([128, JT * SB], F32, tag="ps",
                                       name=f"ps_v{sb}s{stl}")
                    else:
                        ps = ps_mix.tile([128, GJ], F32, tag="mx",
                                         name=f"ps_v{sb}s{stl}")
                    for d in range(DT):
                        nc.tensor.matmul(
                            ps[:, 0:GJ],
                            xv_sb[sb][:, d, stl * 128:(stl + 1) * 128],
                            wv_sb[:, d, :],
                            start=(d == 0),
                            stop=(d == DT - 1),
                        )
                    nc.vector.tensor_copy(
                        vh_sb[:, st, :, 0:64],
                        ps[:, 0:GJ].rearrange("p (h e) -> p h e", h=GH),
                    )
                return u

            # ordered prologue units with force-drain support
            prolog = []
            for sb in range(NCH):
                if sb == 0:
                    # the first units run while the mix pool is still empty
                    # (po claims are lazy) -- keep them off the ps rotation
                    prolog += [("q0j1", kq_unit("q", 0, 1, pool="mix")),
                               ("k0j1", kq_unit("k", 0, 1, pool="mix")),
                               ("v00", v_unit(0, 0, pool="mix"))]
                else:
                    mx = "mix" if sb == 1 else None
                    prolog += [(f"k{sb}j0", kq_unit("k", sb, 0, pool=mx)),
                               (f"v{sb-1}1", v_unit(sb - 1, 1, pool=mx)),
                               (f"v{sb-1}2", v_unit(sb - 1, 2)),
                               (f"k{sb}j1", kq_unit("k", sb, 1)),
                               (f"v{sb-1}3", v_unit(sb - 1, 3)),
                               (f"v{sb}0", v_unit(sb, 0))]
            prolog += [("v31", v_unit(3, 1)), ("v32", v_unit(3, 2)),
                       ("v33", v_unit(3, 3)),
                       ("q1j0", kq_unit("q", 1, 0)), ("q1j1", kq_unit("q", 1, 1))]
            prolog_keys = [k for k, _ in prolog]
            prolog_pos = 0

            def drain_until(key):
                nonlocal prolog_pos
                if key not in prolog_keys:
                    return
                idx = prolog_keys.index(key)
                while prolog_pos <= idx:
                    prolog[prolog_pos][1]()
                    prolog_pos += 1

            def drain_all():
                nonlocal prolog_pos
                while prolog_pos < len(prolog):
                    prolog[prolog_pos][1]()
                    prolog_pos += 1

            def prolog_filler(n=1):
                nonlocal prolog_pos
                for _ in range(n):
                    if prolog_pos < len(prolog):
                        u = prolog[prolog_pos][1]
                        prolog_pos += 1
                        u()

            # ---- stage B ----
            # Scores land as pt[k, q] (two heads of the pair side by side in
            # the 1024-wide exp tile); PV contracts k in the partition dim so
            # each output tile is [q(128), 65] -- free size 65 per matmul,
            # with the ones-column giving the softmax denominator at col 64.
            po_live = {}
            po_pend = {}

            def b_pair(hf, hp, st_lo, st_hi, fillers=None, f_start=2):
                hs = slice(hf * SB, (hf + 1) * SB)
                jt = hp

                def get_po():
                    # lazy: PSUM claimed only at the first PV matmul, which
                    # trails scores/exp by PEND iterations. A matmul with
                    # start=True zeroes the whole PSUM bank, clobbering the
                    # sibling qs-groups sharing it -- so the bank is zeroed
                    # once by DVE and every PV matmul accumulates (start
                    # False).
                    if (hf, hp) not in po_live:
                        tiles = [
                            ps_mix.tile([128, QT, 128], F32, tag="mx",
                                        name=f"po_{hf}_{hp}_{hl}")
                            for hl in range(2)
                        ]
                        for t in tiles:
                            nc.vector.memset(t[:], 0.0)
                        po_live[(hf, hp)] = tiles
                    return po_live[(hf, hp)]

                def pv_mms(st, pt):
                    po = get_po()
                    for hl in range(2):
                        for qs in range(QT):
                            nc.tensor.matmul(
                                po[hl][:, qs, 0:65],
                                pt[:, hl * SB + qs * 128:hl * SB + (qs + 1) * 128],
                                vh_sb[:, st, 2 * hp + hl, :],
                                start=False,
                                stop=(st == ST - 1),
                                skip_group_check=True,
                            )

                pend = po_pend.pop((hf, hp), [])
                for st in range(st_lo, st_hi):
                    ps = ps_s.tile([128, JT * SB], F32, tag="ps",
                                   name=f"psb_{hf}_{hp}_{st}")
                    for hl in range(2):
                        base = 64 * hl
                        nc.tensor.matmul(
                            ps[:, hl * SB:(hl + 1) * SB],
                            khT_sb[base:base + 64, jt, st * 128:(st + 1) * 128],
                            qhT_sb[base:base + 64, jt, hs],
                            start=True, stop=True,
                        )
                    pt = p_pool.tile([128, JT * SB], BF16, tag="pt",
                                     name=f"pt_{hf}_{hp}_{st}")
                    nc.scalar.activation(pt[:], ps[:], AF.Exp, scale=0.125)
                    pend.append((st, pt))
                    if len(pend) > PEND:
                        pv_mms(*pend.pop(0))
                    if fillers and st >= st_lo + f_start:
                        fillers.pop(0)()
                if st_hi < ST:
                    po_pend[(hf, hp)] = pend
                    return []
                for p in pend:
                    pv_mms(*p)
                # normalize: per-partition scalar multiply by 1/den (DVE only)
                po = get_po()
                ops = [o_pool.tile([128, 128], BF16, tag="opair",
                                   name=f"op_{hf}_{hp}_{qs}")
                       for qs in range(QT)]
                for hl in range(2):
                    rcp = s_pool.tile([128, QT], F32, tag="rcp",
                                      name=f"rcp_{hf}_{hp}_{hl}")
                    nc.vector.reciprocal(rcp[:], po[hl][:, :, 64:65])
                    for qs in range(QT):
                        nc.vector.tensor_scalar_mul(
                            ops[qs][:, hl * 64:(hl + 1) * 64],
                            po[hl][:, qs, 0:64],
                            rcp[:, qs:qs + 1],
                        )

                # deferred: PE transpose [q,128] -> [128,q] + DVE copy to oall
                def transpose_unit(qs):
                    def u():
                        tp = ps_mix.tile([128, 128], BF16, tag="mx",
                                         name=f"tp_{hf}_{hp}_{qs}")
                        nc.tensor.transpose(tp[:], ops[qs][:], id_sb[:])
                        nc.vector.tensor_copy(
                            oall_sb[:, jt, hf * SB + qs * 128:
                                    hf * SB + (qs + 1) * 128],
                            tp[:],
                        )
                    return u
                return [transpose_unit(qs) for qs in range(QT)]

            # ---- stage C: output projection for one 512-wide half ----
            yr = yT.rearrange("(t p) s -> t p s", p=128)

            def c_units(hf, last=False):
                hs = slice(hf * SB, (hf + 1) * SB)
                units = []
                for mt in range(DT):
                    def u(mt=mt):
                        pc = ps_mix.tile([128, SB], F32, tag="mx",
                                         name=f"pc_{hf}_{mt}")
                        for kt in range(JT):
                            nc.tensor.matmul(
                                pc[:],
                                wo_sb[:, kt, mt * 128:(mt + 1) * 128],
                                oall_sb[:, kt, hs],
                                start=(kt == 0),
                                stop=(kt == JT - 1),
                            )
                        yt = y_pool.tile([128, SB], BF16, tag="yt",
                                         name=f"yt_{hf}_{mt}")
                        # the scalar engine is free of exps only in the tail
                        if last and mt % 2:
                            nc.scalar.copy(yt[:], pc[:])
                            nc.scalar.dma_start(yr[mt, :, hs], yt[:])
                        else:
                            nc.vector.tensor_copy(yt[:], pc[:])
                            nc.sync.dma_start(yr[mt, :, hs], yt[:])
                    units.append(u)
                return units

            # ---- fused schedule ----
            # Half 0 is interleaved with the projections chunk-wise: scores
            # consume k sk-tiles as each chunk's projection lands, and the
            # remaining projection units spread one-per-iteration as fillers.
            kq_unit("q", 0, 0)()
            kq_unit("k", 0, 0)()
            pending = []
            for sb in range(NCH):
                for hp in range(GH // 2):
                    drain_until(f"k{sb}j{hp}")
                    npop = 2 if sb >= NCH - 2 else 1
                    fl = [lambda n=npop: prolog_filler(n)] * 3
                    pending.extend(
                        b_pair(0, hp, sb * STB, (sb + 1) * STB,
                               fillers=fl, f_start=1)
                    )
                    if sb == NCH - 1 and hp == 0:
                        drain_until("v33")
            for hf in range(1, NCH):
                drain_until(f"q{hf}j0")  # no-op except hf=1 (prolog tail)
                drain_until(f"q{hf}j1")
                cu = c_units(hf - 1)
                fillers = pending + cu[:4]
                t0 = b_pair(hf, 0, 0, ST, fillers)
                fillers2 = fillers + t0 + cu[4:]
                if hf + 1 < NCH:
                    fillers2.append(kq_unit("q", hf + 1, 0, pool="mix"))
                    fillers2.append(kq_unit("q", hf + 1, 1, pool="mix"))
                t1 = b_pair(hf, 1, 0, ST, fillers2)
                for u in fillers2:
                    u()
                pending = t1
            for u in pending:
                u()
            for u in c_units(NCH - 1, last=True):
                u()

    nc.compile()
    return nc


_NC_CACHE = {}


def _get_nc(S=S_FULL):
    if S not in _NC_CACHE:
        _NC_CACHE[S] = build_nc(S)
    return _NC_CACHE[S]


def _bf16(x):
    import ml_dtypes
    return np.asarray(x).astype(ml_dtypes.bfloat16)


def make_in_maps(q, k, v, Wq, bq, Wk, bk, Wv, bv, Wo, bo, S=S_FULL):
    q = np.asarray(q, np.float32)
    k = np.asarray(k, np.float32)
    v = np.asarray(v, np.float32)
    Wq = np.asarray(Wq, np.float32)
    Wk = np.asarray(Wk, np.float32)
    Wv = np.asarray(Wv, np.float32)
    Wo = np.asarray(Wo, np.float32)
    bq = np.asarray(bq, np.float32)
    bk = np.asarray(bk, np.float32)

    SB = 512
    NCH = S // SB
    DT = D_MODEL // 128

    def xtile(x):
        # [S, D] -> xT [D, S] -> [NCH, 128, DT, SB]: t[sb, p, d, s] = x[sb*SB+s, d*128+p]
        xT = x.T  # [D, S]
        return np.ascontiguousarray(
            _bf16(xT.reshape(DT, 128, NCH, SB).transpose(2, 1, 0, 3))
        )

    def wtile(wT):
        # [D, GJ] -> [128, DT, GJ]
        return np.ascontiguousarray(_bf16(wT.reshape(DT, 128, GJ).transpose(1, 0, 2)))

    ident = np.ascontiguousarray(_bf16(np.eye(128, dtype=np.float32)))
    in_maps = []
    for c in range(N_CORES):
        b, g = divmod(c, GH)
        sl = slice(g * GJ, (g + 1) * GJ)
        woT = Wo[:, sl].T  # [GJ, D]
        in_maps.append({
            "xqT": xtile(q[b, :S]),
            "xkT": xtile(k[b, :S]),
            "xvT": xtile(v[b, :S]),
            "wqT": wtile(Wq[sl].T),
            "wkT": wtile(Wk[sl].T),
            "wvT": wtile(Wv[sl].T),
            "woT": np.ascontiguousarray(
                _bf16(woT.reshape(2, 128, D_MODEL).transpose(1, 0, 2))
            ),
            "ident": ident,
            "bq": np.ascontiguousarray(bq[sl].reshape(2, 128).T),
            "bk": np.ascontiguousarray(bk[sl].reshape(2, 128).T),
        })
    return in_maps


def gather_out(results, Wo, bv, bo, S=S_FULL):
    Wo = np.asarray(Wo, np.float32)
    bv = np.asarray(bv, np.float32)
    bo = np.asarray(bo, np.float32)
    out = np.zeros((B, S, D_MODEL), np.float32)
    for c in range(N_CORES):
        out[c // GH] += results[c]["yT"].astype(np.float32).T
    out += bo + Wo @ bv
    return out


def kernel(q, k, v, Wq, bq, Wk, bk, Wv, bv, Wo, bo):
    from concourse.bass_utils import run_bass_kernel_spmd

    nc = _get_nc(S_FULL)
    in_maps = make_in_maps(q, k, v, Wq, bq, Wk, bk, Wv, bv, Wo, bo)
    res = run_bass_kernel_spmd(nc, in_maps, core_ids=list(range(N_CORES)))
    return gather_out(res.results, Wo, bv, bo)
